# revision 39
# baseline (speedup 1.0000x reference)
"""Multi-head causal self-attention on 8 Trainium2 NeuronCores.

Sharding: core c -> (batch b = c//2, head-group hg = c%2): data-parallel over
the 4 batches x tensor-parallel over 2 groups of 8 heads. c_attn is
column-parallel, fc_out row-parallel (Megatron); the row-parallel partial sums
are reduced on the host during the gather/unshard step (fp16 partials,
fp32 accumulation).

All-16-bit compute with host-side pre-transpose of x:
 - host ships x^T/weights as fp16, weights pre-shuffled to [128, chunks*cols]
   so every weight-DMA partition line is >= 4KB contiguous (full HBM rate)
 - Q/K fp16; exp output + V in bf16 (range covers exp(q.q/8) diagonal tails)
 - softmax denominators fused into the PV matmul via a ones-column on V
 - "superstream" energy: the two heads of a pair share each eps PSUM tile
   (partition halves 0:64 / 64:128 = disjoint PE row groups), so their K=64
   energy matmuls co-issue in the array (133 vs 229 ns/MM measured); eps
   pools ping-pong per k-tile to keep ScalarE's exp stream fed
 - q-major PV: per (head, q-chunk of 128) a K=128 M=128 N=65 matmul with the
   exp output as the stationary accumulates O[q, d|denom] in PSUM (42 vs 229
   ns/MM): the PE streams 65 columns instead of 512 for 65 useful rows.
   PSUM start=True zeroes a whole 2KB bank ("zero region"), so exactly one
   start/stop per O bank per q-window; sub-regions ride the bank-wide zero
 - softmax normalize = per-partition reciprocal_approx_fast + scalar multiply
   (no PE work); A reaches fc's AT layout via XBAR DMA transposes (DMA idle)
 - decoupled attention pipeline: exp outputs land in a 2-deep (pair-parity)
   PT ring; each pair's PV sweep runs as a *filler stream* woven into the
   NEXT pair's ACT-bound energy/exp stretch, alongside QKV projection of
   s-block st+1 and the deferred fc_out matmuls (weighted pacing, ~5x cost
   ratio big/small matmuls), so the (in-order) PE queue never parks behind
   an exp. Pair g's sweep is fully emitted before pair g+2's energies, which
   makes the PT ring race-free by PE queue order alone.
"""
import numpy as np
from collections import deque
from contextlib import ExitStack

import concourse.bass as bass
import concourse.mybir as mybir
import concourse.tile as tile
from concourse import bacc
from concourse.bass_utils import run_bass_kernel_spmd

dt = mybir.dt
AF = mybir.ActivationFunctionType

B, S, E, H = 4, 2048, 1024, 16
D = 64            # head dim
HL = 8            # heads per core
DL = HL * D       # 512, local attention width
ECH = E // 128    # 8 contraction chunks over embed dim
NQT = S // 512    # 4 q-tiles of 512
NST = S // 128    # 16 s-subtiles of 128
SCALE = 1.0 / np.sqrt(np.float32(D))

_CACHE = {}
SERIAL_SWEEP = False


def _build(reps=1, loop=1, upto=3, act_slim=False, skip_affsel=False,
           skip_norm=False, skip_energy=False, skip_pv=False,
           skip_exp=False, norm_mode="full"):
    nc = bacc.Bacc("TRN2")
    f16, bf16, f32 = dt.float16, dt.bfloat16, dt.float32

    xT = nc.dram_tensor("xT", [E, S], f16, kind="ExternalInput")
    wq = nc.dram_tensor("wq", [128, ECH * DL], f16, kind="ExternalInput")
    wk = nc.dram_tensor("wk", [128, ECH * DL], f16, kind="ExternalInput")
    wv = nc.dram_tensor("wv", [128, ECH * DL], f16, kind="ExternalInput")
    wo = nc.dram_tensor("wo", [128, 4 * E], f16, kind="ExternalInput")
    bqk = nc.dram_tensor("bqk", [2 * DL], f32, kind="ExternalInput")
    bv = nc.dram_tensor("bv", [DL], f32, kind="ExternalInput")
    bo = nc.dram_tensor("bo", [E], f32, kind="ExternalInput")
    out = nc.dram_tensor("out", [S, E], f16, kind="ExternalOutput")

    def bcast_dram(row_ap, parts):
        return bass.AP(tensor=row_ap.tensor, offset=row_ap.offset,
                       ap=[[0, parts]] + list(row_ap.ap[1:]))

    with tile.TileContext(nc) as tc, ExitStack() as top:
        top.enter_context(nc.allow_low_precision(
            reason="16-bit attention compute is intentional"))
        persist = top.enter_context(tc.tile_pool(name="persist", bufs=1))

        # QT/KT: [d, s] pair-packed fp16: pair p=(head 2p, 2p+1) -> partitions
        # (0:64, 64:128), free block p*2048 + s
        QT = persist.tile([128, 4 * S], f16)
        KT = persist.tile([128, 4 * S], f16)
        AT = persist.tile([128, 4 * S], f16)
        # V: [s, d] bf16 per (head l, s-subtile t): free (l*16+t)*65,
        # cols 0:64 = V, col 64 = 1.0 (fused softmax denominator)
        V = persist.tile([128, HL * NST * 65], bf16)
        # consts: [0:128) ones, [128:136) bqk, [136:648) bv bcast
        consts = persist.tile([128, 648], f32)
        ones_f = consts[:, 0:128]
        nc.vector.memset(ones_f, 1.0)
        bqk_sb = consts[:, 128:136]
        nc.sync.dma_start(out=bqk_sb, in_=bqk.rearrange("(c p) -> p c", p=128))
        bv_bc = consts[:, 136:648]
        nc.sync.dma_start(out=bv_bc, in_=bcast_dram(bv[None, :], 128))

        def _rep_body():
            ctx = ExitStack()
            pw = ctx.enter_context(tc.tile_pool(name="pw", bufs=1))
            p_s = ctx.enter_context(tc.tile_pool(name="p_s", bufs=2))
            p_o = ctx.enter_context(tc.tile_pool(name="p_o", bufs=2))
            ps_e0 = ctx.enter_context(
                tc.tile_pool(name="ps_e0", bufs=1, space="PSUM"))
            ps_e1 = ctx.enter_context(
                tc.tile_pool(name="ps_e1", bufs=1, space="PSUM"))
            ps_o = ctx.enter_context(
                tc.tile_pool(name="ps_o", bufs=2, space="PSUM"))
            ps_x = ctx.enter_context(
                tc.tile_pool(name="ps_x", bufs=2, space="PSUM"))

            xT_sb = pw.tile([128, ECH, S], f16)
            # PT ping-pong (by global pair parity): [128 k, 16 kt, 2 h,
            # 512 q] bf16 exp outputs. pair g writes/reads PTs[g%2]; since
            # pair g's sweep is fully emitted before pair g+2's energies
            # enter the (in-order) PE queue, the overwrite is race-free by
            # construction.
            PT0 = pw.tile([128, NST, 2, 512], bf16)
            PT1 = pw.tile([128, NST, 2, 512], bf16)
            PTs = [PT0, PT1]
            if skip_exp:
                # timing-ablation: PT must still be written before PV reads
                nc.vector.memset(PT0[:], 0.001)
                nc.vector.memset(PT1[:], 0.001)
            wq_sb = pw.tile([128, ECH, DL], f16)
            wk_sb = pw.tile([128, ECH, DL], f16)
            wv_sb = pw.tile([128, ECH, DL], f16)
            wo_sb = pw.tile([128, 4, E], f16)
            bo_bc = pw.tile([128, E], f32)

            # weights are host-pre-shuffled to [128, chunks*cols] so every
            # DMA partition line is >= 4KB contiguous (full HBM throughput)
            xTr = xT.rearrange("(eo p) s -> p eo s", p=128)
            # wq/xT0 feed the very first proj chain: split them across 4
            # DMA queues each so they take a ~8/14 share of HBM bandwidth
            # instead of 2/9 while the bulk input DMAs stream alongside
            wqr = wq.rearrange("p (eo d) -> p eo d", eo=ECH)
            for eh in range(4):
                nc.sync.dma_start(out=wq_sb[:, 2 * eh:2 * eh + 2, :],
                                  in_=wqr[:, 2 * eh:2 * eh + 2, :])
            for eh in range(4):
                nc.sync.dma_start(out=xT_sb[:, 2 * eh:2 * eh + 2, 0:512],
                                  in_=xTr[:, 2 * eh:2 * eh + 2, 0:512])
            nc.sync.dma_start(out=wk_sb[:],
                              in_=wk.rearrange("p (eo d) -> p eo d", eo=ECH))
            nc.sync.dma_start(out=wv_sb[:],
                              in_=wv.rearrange("p (eo d) -> p eo d", eo=ECH))
            for st in range(1, NQT):
                nc.sync.dma_start(out=xT_sb[:, :, st * 512:(st + 1) * 512],
                                  in_=xTr[:, :, st * 512:(st + 1) * 512])
            nc.sync.dma_start(out=wo_sb[:],
                              in_=wo.rearrange("p (co n) -> p co n", co=4))
            nc.sync.dma_start(out=bo_bc[:], in_=bcast_dram(bo[None, :], 128))

            Vv = V[:].rearrange("p (l t c) -> p l t c", l=HL, c=65)
            eps_pools = [ps_e0, ps_e1]
            eps_fresh = [2, 2]  # first-use garbage memsets per pool

            def proj_stream(st, part=None):
                """QKV projections for s-window st. Yields after each MM.
                `part` selects ('qk', d) / ('v', s) chunks, so window 0 can
                emit just pair-0's Q/K + all V up front and defer the rest
                into the qt=0 attention stretch as filler."""

                def qk_chunk(dch):
                    w_sb = wq_sb if dch < 4 else wk_sb
                    dsl = slice((dch % 4) * 128, (dch % 4) * 128 + 128)
                    pq = ps_x.tile([128, 512], f32, tag="px", name="pq")
                    for ech in range(ECH):
                        nc.tensor.matmul(
                            pq[:], w_sb[:, ech, dsl],
                            xT_sb[:, ech, st * 512:(st + 1) * 512],
                            start=(ech == 0), stop=(ech == ECH - 1))
                    dest = QT if dch < 4 else KT
                    pair = dch % 4
                    nc.vector.tensor_scalar_add(
                        out=dest[:, pair * S + st * 512:
                                 pair * S + (st + 1) * 512],
                        in0=pq[:], scalar1=bqk_sb[:, dch:dch + 1])
                    yield 40

                def v_chunk(sub):
                    t = st * 4 + sub
                    pv = ps_x.tile([128, 512], f32, tag="px", name="pv")
                    for ech in range(ECH):
                        nc.tensor.matmul(
                            pv[:], xT_sb[:, ech, t * 128:(t + 1) * 128],
                            wv_sb[:, ech, :],
                            start=(ech == 0), stop=(ech == ECH - 1))
                    nc.vector.tensor_add(
                        out=Vv[:, :, t, 0:64],
                        in0=pv[:].rearrange("p (l d) -> p l d", d=64),
                        in1=bv_bc.rearrange("p (l d) -> p l d", d=64))
                    nc.vector.tensor_copy(out=Vv[:, :, t, 64],
                                          in_=ones_f[:, 0:HL])
                    yield 40

                if part is None:
                    part = [("qk", d) for d in range(8)] + \
                           [("v", s) for s in range(4)]
                for kind, i in part:
                    yield from (qk_chunk(i) if kind == "qk" else v_chunk(i))
                yield 1

            def phase1(lp, qt):
                PT = PTs[(qt * 4 + lp) % 2]
                """Energy + exp for head pair lp (heads 2lp, 2lp+1) over one
                q-window. The two heads' energy matmuls share one eps tile
                (rows 0:64 / 64:128 -> disjoint PE row-groups, so they run
                concurrently in the array; 133 vs 229 ns/MM measured); eps
                pools ping-pong per k-tile. exp lands in the persistent PT
                ring (slot = kt), consumed later by this pair's PV sweep,
                which weaves into the NEXT pair's exp stretch as filler —
                so the (in-order) PE queue never parks behind an exp."""
                pair = lp
                q0 = pair * S + qt * 512
                n_kt = 4 * (qt + 1)
                for kt in range(n_kt):
                    pool = eps_pools[kt % 2]
                    eps = pool.tile([128, 2, 512], f32, tag=f"eps{kt % 2}")
                    if eps_fresh[kt % 2] > 0:
                        nc.vector.memset(eps[:], 0.0)
                        eps_fresh[kt % 2] -= 1
                    dj = kt - 4 * qt  # diagonal sub-block index (if >= 0)
                    a = dj * 128 if dj >= 0 else 0
                    if not skip_energy:
                        for h in range(2):
                            pb = h * 64
                            nc.tensor.matmul(
                                eps[:, h, a:],
                                KT[pb:pb + 64, pair * S + kt * 128:
                                   pair * S + (kt + 1) * 128],
                                QT[pb:pb + 64, q0 + a:q0 + 512],
                                start=True, stop=True)
                    yield
                    ao = 448 if act_slim else a
                    if not skip_exp:
                        nc.scalar.activation(out=PT[:, kt, :, ao:],
                                             in_=eps[:, :, ao:],
                                             func=AF.Exp, scale=float(SCALE))
                    if dj >= 0 and not skip_affsel:
                        # keep where q_local - k_local >= 0; only the 128-wide
                        # diagonal block needs masking, both heads share it
                        nc.gpsimd.affine_select(
                            out=PT[:, kt, :, a:a + 128],
                            in_=PT[:, kt, :, a:a + 128],
                            compare_op=mybir.AluOpType.is_ge,
                            fill=0.0, base=0,
                            pattern=[[0, 2], [1, 128]],
                            channel_multiplier=-1)
                    yield

            def pv_sweep(lp, qt):
                """PV + softmax-normalize for pair lp of q-window qt,
                consuming the PT ring kt-outer (so later writers of each PT
                slot unblock early). Per (head, q-chunk of 128): a K=128
                M=128 N=65 matmul with PT as the stationary -> O[q, d|denom]
                accumulates in PSUM (42 vs 229 ns/MM measured). Softmax
                normalize is a per-partition scalar multiply (no PE), and A
                reaches AT-layout via XBAR DMA transposes (DMA is idle)."""
                PT = PTs[(qt * 4 + lp) % 2]
                pair = lp
                q0 = pair * S + qt * 512
                n_kt = 4 * (qt + 1)
                O0 = ps_o.tile([128, 4, 65], f32, tag="oT", name="O0")
                O1 = ps_o.tile([128, 4, 65], f32, tag="oT", name="O1")
                Os = [O0, O1]
                for kt in range(n_kt):
                    dj = kt - 4 * qt
                    for h in range(2):
                        l = 2 * lp + h
                        if skip_pv:
                            yield 1
                            continue
                        for qc in range(4):
                            if dj > qc:
                                continue  # q-chunk fully masked for this kt
                            # start=True zeroes the whole 2KB PSUM bank (the
                            # "zero region"), so exactly one start (first
                            # matmul into the bank) and one stop (last) per
                            # q-window; the other sub-regions accumulate onto
                            # the bank-wide lazy zero.
                            nc.tensor.matmul(
                                Os[h][:, qc, :],
                                PT[:, kt, h, qc * 128:(qc + 1) * 128],
                                V[:, (l * NST + kt) * 65:
                                  (l * NST + kt) * 65 + 65],
                                start=(kt == 0 and qc == 0),
                                stop=(kt == n_kt - 1 and qc == 3))
                            yield 1
                # softmax normalization: O[:, qc, 64] = denominators, per-q on
                # partitions -> plain per-partition scalar ops, no PE needed.
                if skip_norm:
                    yield 1
                    return
                den = p_s.tile([128, 2, 4], f32, tag="den")
                for h in range(2):
                    nc.vector.tensor_copy(out=den[:, h, :],
                                          in_=Os[h][:, :, 64])
                rcp = p_s.tile([128, 2, 4], f32, tag="rcp")
                nc.vector.reciprocal_approx_fast(out=rcp[:], in_=den[:])
                asb = p_s.tile([128, 4, 2, 64], f16, tag="asb")
                for h in range(2):
                    nc.vector.tensor_mul(
                        out=asb[:, :, h, :], in0=Os[h][:, :, 0:64],
                        in1=rcp[:, h, :, None].broadcast_to((128, 4, 64)))
                    yield 1
                if norm_mode == "noat":
                    yield 1
                    return
                for qc in range(4):
                    # XBAR transpose [128 q, 128 dl] -> AT[dl, q] on the DMA
                    nc.sync.dma_start(
                        out=AT[:, q0 + qc * 128:q0 + (qc + 1) * 128],
                        in_=asb[:, qc, :, :], transpose=True)
                yield 1

            def fc_stream(qt):
                """fc_out for q-window qt (row-parallel partial) + out DMA."""
                for st_loc in range(4):
                    st = qt * 4 + st_loc
                    o_sb = p_o.tile([128, E], f16, tag="o_sb")
                    for half in range(2):
                        pf = ps_x.tile([128, 512], f32, tag="px")
                        for dch in range(4):
                            nc.tensor.matmul(
                                pf[:],
                                AT[:, dch * S + st * 128:
                                   dch * S + (st + 1) * 128],
                                wo_sb[:, dch, half * 512:(half + 1) * 512],
                                start=(dch == 0), stop=(dch == 3))
                        nc.vector.tensor_add(
                            out=o_sb[:, half * 512:(half + 1) * 512],
                            in0=pf[:],
                            in1=bo_bc[:, half * 512:(half + 1) * 512])
                        yield 20
                    nc.sync.dma_start(out=out[st * 128:(st + 1) * 128, :],
                                      in_=o_sb[:])
                    yield 1

            PROJ_STEPS = 8 * ECH + 4 * ECH + 1   # 97
            FC_STEPS = 4 * (2 * 4 + 1)           # 36
            LEAD = [("qk", 0), ("qk", 4),
                    ("v", 0), ("v", 1), ("v", 2), ("v", 3)]
            PAIRP = {1: [("qk", 1), ("qk", 5)], 2: [("qk", 2), ("qk", 6)],
                     3: [("qk", 3), ("qk", 7)]}
            proj_done = set()

            def tracked(gen, key):
                yield from gen
                proj_done.add(key)

            # lead-in: only pair-0's Q/K and V for s-window 0; the other
            # pairs' Q/K weave into the qt=0 attention stretch below
            for _ in proj_stream(0, LEAD):
                pass

            if upto < 2:
                for lp in (1, 2, 3):
                    for _ in proj_stream(0, PAIRP[lp]):
                        pass
                for st in range(1, NQT):
                    for _ in proj_stream(st):
                        pass
                ctx.close()
                return

            fillers = deque()
            cur_sweep = [None]   # at most ONE sweep in flight (O-pool order)
            W_BIG = 5            # ~229ns proj/fc matmul vs ~42ns sweep matmul

            def consume_one():
                """Run one filler step; the in-flight sweep has priority
                (PT-ring consumers must not lag behind exp writers).
                Returns the step's pacing weight (0 = nothing left)."""
                if cur_sweep[0] is not None:
                    w = next(cur_sweep[0], "end")
                    if w != "end":
                        return w
                    cur_sweep[0] = None
                while fillers:
                    w = next(fillers[0], "end")
                    if w == "end":
                        fillers.popleft()
                    else:
                        return w
                return 0

            prev_sweep_steps = 0
            for qt in range(NQT):
                n_kt = 4 * (qt + 1)
                sweep_steps = 2 * (16 * qt + 10) + 3
                budget = prev_sweep_steps if cur_sweep[0] is not None else 0
                if qt == 0:
                    for lp2 in (1, 2, 3):
                        fillers.append(tracked(proj_stream(0, PAIRP[lp2]),
                                               (0, lp2)))
                        budget += 2 * 40 + 1
                if qt + 1 < NQT:
                    # next window's pair-0 prefix (Q/K pair0 + all V) must
                    # finish by this window's end; the other pairs' chunks
                    # may spill into the next window, gated per pair below
                    fillers.append(tracked(proj_stream(qt + 1, LEAD),
                                           (qt + 1, 0)))
                    budget += 6 * 40 + 1
                    for lp2 in (1, 2, 3):
                        fillers.append(tracked(proj_stream(qt + 1,
                                                           PAIRP[lp2]),
                                               (qt + 1, lp2)))
                        budget += 2 * 40 + 1
                fc_stage = deque()
                if qt == NQT - 1 and upto >= 3:
                    # one fc stream per pair-stretch, so filler supply lasts
                    # through the whole (longest) window instead of running
                    # dry before the final pair's ACT-bound stretch
                    fc_stage = deque(fc_stream(q2) for q2 in range(NQT - 1))
                total_rounds = 4 * 2 * n_kt
                r = 0
                done_f = 0
                for lp in range(4):
                    if fc_stage and lp >= 1:
                        fillers.append(fc_stage.popleft())
                        budget += FC_STEPS * 5
                    # correctness gate: this pair's Q/K chunks must be fully
                    # emitted before its energy matmuls
                    while lp > 0 and (qt, lp) not in proj_done:
                        if not consume_one():
                            break
                    for _ in phase1(lp, qt):
                        r += 1
                        target = min(budget, budget * (r + 2) // total_rounds)
                        while done_f < target:
                            w = consume_one()
                            if w == 0:
                                break
                            done_f += w
                    # install this pair's sweep (for the next exp stretch);
                    # finish any unconsumed previous sweep first
                    if cur_sweep[0] is not None:
                        while next(cur_sweep[0], "end") != "end":
                            pass
                    cur_sweep[0] = pv_sweep(lp, qt)
                    if SERIAL_SWEEP:
                        while next(cur_sweep[0], "end") != "end":
                            pass
                        cur_sweep[0] = None
                    budget += sweep_steps
                # drain only through the next window's pair-0 prefix
                # (Q/K pair0 + V); later pairs' proj chunks keep weaving
                # into the next window's stretches, gated per pair above
                while qt + 1 < NQT and (qt + 1, 0) not in proj_done:
                    if not consume_one():
                        break
                if qt + 1 == NQT:
                    while fillers:
                        if next(fillers[0], "end") == "end":
                            fillers.popleft()
                prev_sweep_steps = sweep_steps
            for g in fc_stage:
                fillers.append(g)
            while consume_one():
                pass
            if upto >= 3:
                for _ in fc_stream(NQT - 1):
                    pass
            ctx.close()

        if loop > 1:
            with tc.For_i(0, loop, 1):
                _rep_body()
        else:
            for _rep in range(reps):
                _rep_body()

    nc.finalize()
    return nc


def _in_maps(x, w_attn, b_attn, w_out, b_out):
    x = np.asarray(x, np.float32)
    w_attn = np.asarray(w_attn, np.float32)
    b_attn = np.asarray(b_attn, np.float32)
    w_out = np.asarray(w_out, np.float32)
    b_out = np.asarray(b_out, np.float32)
    zeros_e = np.zeros((E,), np.float32)

    def chunk_rows(w):
        # [(nch*128), cols] -> [128, nch*cols]: row p holds chunks eo of
        # original rows eo*128+p, so each DMA partition line is contiguous
        nch = w.shape[0] // 128
        return np.ascontiguousarray(
            w.reshape(nch, 128, w.shape[1]).transpose(1, 0, 2)
            .reshape(128, nch * w.shape[1]).astype(np.float16))

    maps = []
    for c in range(8):
        b, hg = c // 2, c % 2
        sq = slice(0 * E + hg * DL, 0 * E + (hg + 1) * DL)
        sk = slice(1 * E + hg * DL, 1 * E + (hg + 1) * DL)
        sv = slice(2 * E + hg * DL, 2 * E + (hg + 1) * DL)
        maps.append({
            "xT": np.ascontiguousarray(x[b].T.astype(np.float16)),
            "wq": chunk_rows(w_attn[:, sq]),
            "wk": chunk_rows(w_attn[:, sk]),
            "wv": chunk_rows(w_attn[:, sv]),
            "wo": chunk_rows(w_out[hg * DL:(hg + 1) * DL, :]),
            "bqk": np.concatenate([b_attn[sq], b_attn[sk]]),
            "bv": np.ascontiguousarray(b_attn[sv]),
            "bo": b_out if hg == 0 else zeros_e,
        })
    return maps


def _run(x, w_attn, b_attn, w_out, b_out, trace=False):
    if "nc" not in _CACHE:
        _CACHE["nc"] = _build()
    maps = _in_maps(x, w_attn, b_attn, w_out, b_out)
    res = run_bass_kernel_spmd(_CACHE["nc"], maps, list(range(8)), trace=trace)
    outs = np.empty((B, S, E), np.float32)
    for b in range(B):
        outs[b] = (res.results[2 * b]["out"].astype(np.float32)
                   + res.results[2 * b + 1]["out"].astype(np.float32))
    return outs, res


def kernel(x, w_attn, b_attn, w_out, b_out):
    outs, _ = _run(x, w_attn, b_attn, w_out, b_out, trace=False)
    return outs



# revision 40
# speedup vs baseline: 1.1534x; 1.1534x over previous
"""Multi-head causal self-attention on 8 Trainium2 NeuronCores.

Sharding: core c -> (batch b = c//2, head-group hg = c%2): data-parallel over
the 4 batches x tensor-parallel over 2 groups of 8 heads. c_attn is
column-parallel, fc_out row-parallel (Megatron); the row-parallel partial sums
are reduced on the host during the gather/unshard step (fp16 partials,
fp32 accumulation).

All-16-bit compute with host-side pre-transpose of x:
 - host ships x^T/weights as fp16, weights pre-shuffled to [128, chunks*cols]
   so every weight-DMA partition line is >= 4KB contiguous (full HBM rate)
 - Q/K fp16; exp output + V in bf16 (range covers exp(q.q/8) diagonal tails)
 - softmax denominators fused into the PV matmul via a ones-column on V
 - "superstream" energy: the two heads of a pair share each eps PSUM tile
   (partition halves 0:64 / 64:128 = disjoint PE row groups), so their K=64
   energy matmuls co-issue in the array (133 vs 229 ns/MM measured); eps
   pools ping-pong per k-tile to keep ScalarE's exp stream fed
 - q-major PV: per (head, q-chunk of 128) a K=128 M=128 N=65 matmul with the
   exp output as the stationary accumulates O[q, d|denom] in PSUM (42 vs 229
   ns/MM): the PE streams 65 columns instead of 512 for 65 useful rows.
   PSUM start=True zeroes a whole 2KB bank ("zero region"), so exactly one
   start/stop per O bank per q-window; sub-regions ride the bank-wide zero
 - softmax normalize = per-partition reciprocal_approx_fast + scalar multiply
   (no PE work); A reaches fc's AT layout via XBAR DMA transposes (DMA idle)
 - decoupled attention pipeline: exp outputs land in a 2-deep (pair-parity)
   PT ring; each pair's PV sweep runs as a *filler stream* woven into the
   NEXT pair's ACT-bound energy/exp stretch, alongside QKV projection of
   s-block st+1 and the deferred fc_out matmuls (weighted pacing, ~5x cost
   ratio big/small matmuls), so the (in-order) PE queue never parks behind
   an exp. Pair g's sweep is fully emitted before pair g+2's energies, which
   makes the PT ring race-free by PE queue order alone.
"""
import numpy as np
from collections import deque
from contextlib import ExitStack

import concourse.bass as bass
import concourse.mybir as mybir
import concourse.tile as tile
from concourse import bacc
from concourse.bass_utils import run_bass_kernel_spmd

dt = mybir.dt
AF = mybir.ActivationFunctionType

B, S, E, H = 4, 2048, 1024, 16
D = 64            # head dim
HL = 8            # heads per core
DL = HL * D       # 512, local attention width
ECH = E // 128    # 8 contraction chunks over embed dim
NQT = S // 512    # 4 q-tiles of 512
NST = S // 128    # 16 s-subtiles of 128
SCALE = 1.0 / np.sqrt(np.float32(D))

_CACHE = {}
SERIAL_SWEEP = False


def _build(reps=1, loop=1, upto=3, act_slim=False, skip_affsel=False,
           skip_norm=False, skip_energy=False, skip_pv=False,
           skip_exp=False, norm_mode="full"):
    nc = bacc.Bacc("TRN2")
    f16, bf16, f32 = dt.float16, dt.bfloat16, dt.float32

    xT = nc.dram_tensor("xT", [E, S], f16, kind="ExternalInput")
    wq = nc.dram_tensor("wq", [128, ECH * DL], f16, kind="ExternalInput")
    wk = nc.dram_tensor("wk", [128, ECH * DL], f16, kind="ExternalInput")
    wv = nc.dram_tensor("wv", [128, ECH * DL], f16, kind="ExternalInput")
    wo = nc.dram_tensor("wo", [128, 4 * E], f16, kind="ExternalInput")
    bqk = nc.dram_tensor("bqk", [2 * DL], f32, kind="ExternalInput")
    bv = nc.dram_tensor("bv", [DL], f32, kind="ExternalInput")
    bo = nc.dram_tensor("bo", [E], f32, kind="ExternalInput")
    out = nc.dram_tensor("out", [S, E], f16, kind="ExternalOutput")

    def bcast_dram(row_ap, parts):
        return bass.AP(tensor=row_ap.tensor, offset=row_ap.offset,
                       ap=[[0, parts]] + list(row_ap.ap[1:]))

    with tile.TileContext(nc) as tc, ExitStack() as top:
        top.enter_context(nc.allow_low_precision(
            reason="16-bit attention compute is intentional"))
        persist = top.enter_context(tc.tile_pool(name="persist", bufs=1))

        # QT/KT: [d, s] pair-packed fp16: pair p=(head 2p, 2p+1) -> partitions
        # (0:64, 64:128), free block p*2048 + s
        QT = persist.tile([128, 4 * S], f16)
        KT = persist.tile([128, 4 * S], f16)
        AT = persist.tile([128, 4 * S], f16)
        # V: [s, d] bf16 per (head l, s-subtile t): free (l*16+t)*65,
        # cols 0:64 = V, col 64 = 1.0 (fused softmax denominator)
        V = persist.tile([128, HL * NST * 65], bf16)
        # consts: [0:128) ones, [128:136) bqk, [136:648) bv bcast
        consts = persist.tile([128, 648], f32)
        ones_f = consts[:, 0:128]
        nc.vector.memset(ones_f, 1.0)
        bqk_sb = consts[:, 128:136]
        nc.sync.dma_start(out=bqk_sb, in_=bqk.rearrange("(c p) -> p c", p=128))
        bv_bc = consts[:, 136:648]
        nc.sync.dma_start(out=bv_bc, in_=bcast_dram(bv[None, :], 128))

        def _rep_body():
            ctx = ExitStack()
            pw = ctx.enter_context(tc.tile_pool(name="pw", bufs=1))
            p_s = ctx.enter_context(tc.tile_pool(name="p_s", bufs=2))
            p_o = ctx.enter_context(tc.tile_pool(name="p_o", bufs=2))
            ps_e0 = ctx.enter_context(
                tc.tile_pool(name="ps_e0", bufs=1, space="PSUM"))
            ps_e1 = ctx.enter_context(
                tc.tile_pool(name="ps_e1", bufs=1, space="PSUM"))
            ps_o = ctx.enter_context(
                tc.tile_pool(name="ps_o", bufs=2, space="PSUM"))
            ps_x = ctx.enter_context(
                tc.tile_pool(name="ps_x", bufs=2, space="PSUM"))

            xT_sb = pw.tile([128, ECH, S], f16)
            # PT ping-pong (by global pair parity): [128 k, 16 kt, 2 h,
            # 512 q] bf16 exp outputs. pair g writes/reads PTs[g%2]; since
            # pair g's sweep is fully emitted before pair g+2's energies
            # enter the (in-order) PE queue, the overwrite is race-free by
            # construction.
            PT0 = pw.tile([128, NST, 2, 512], bf16)
            PT1 = pw.tile([128, NST, 2, 512], bf16)
            PTs = [PT0, PT1]
            if skip_exp:
                # timing-ablation: PT must still be written before PV reads
                nc.vector.memset(PT0[:], 0.001)
                nc.vector.memset(PT1[:], 0.001)
            wq_sb = pw.tile([128, ECH, DL], f16)
            wk_sb = pw.tile([128, ECH, DL], f16)
            wv_sb = pw.tile([128, ECH, DL], f16)
            wo_sb = pw.tile([128, 4, E], f16)
            bo_bc = pw.tile([128, E], f32)

            # weights are host-pre-shuffled to [128, chunks*cols] so every
            # DMA partition line is >= 4KB contiguous (full HBM throughput)
            xTr = xT.rearrange("(eo p) s -> p eo s", p=128)
            # wq/xT0 feed the very first proj chain: split them across 4
            # DMA queues each so they take a ~8/14 share of HBM bandwidth
            # instead of 2/9 while the bulk input DMAs stream alongside
            wqr = wq.rearrange("p (eo d) -> p eo d", eo=ECH)
            for eh in range(4):
                nc.sync.dma_start(out=wq_sb[:, 2 * eh:2 * eh + 2, :],
                                  in_=wqr[:, 2 * eh:2 * eh + 2, :])
            for eh in range(4):
                nc.sync.dma_start(out=xT_sb[:, 2 * eh:2 * eh + 2, 0:512],
                                  in_=xTr[:, 2 * eh:2 * eh + 2, 0:512])
            nc.sync.dma_start(out=wk_sb[:],
                              in_=wk.rearrange("p (eo d) -> p eo d", eo=ECH))
            nc.sync.dma_start(out=wv_sb[:],
                              in_=wv.rearrange("p (eo d) -> p eo d", eo=ECH))
            for st in range(1, NQT):
                nc.sync.dma_start(out=xT_sb[:, :, st * 512:(st + 1) * 512],
                                  in_=xTr[:, :, st * 512:(st + 1) * 512])
            nc.sync.dma_start(out=wo_sb[:],
                              in_=wo.rearrange("p (co n) -> p co n", co=4))
            nc.sync.dma_start(out=bo_bc[:], in_=bcast_dram(bo[None, :], 128))

            Vv = V[:].rearrange("p (l t c) -> p l t c", l=HL, c=65)
            eps_pools = [ps_e0, ps_e1]
            eps_fresh = [0, 0]  # subtile deps cover first uses

            def proj_stream(st, part=None):
                """QKV projections for s-window st. Yields after each MM.
                `part` selects ('qk', d) / ('v', s) chunks, so window 0 can
                emit just pair-0's Q/K + all V up front and defer the rest
                into the qt=0 attention stretch as filler."""

                def qk_chunk(dch):
                    w_sb = wq_sb if dch < 4 else wk_sb
                    dsl = slice((dch % 4) * 128, (dch % 4) * 128 + 128)
                    pq = ps_x.tile([128, 512], f32, tag="px", name="pq")
                    for ech in range(ECH):
                        nc.tensor.matmul(
                            pq[:], w_sb[:, ech, dsl],
                            xT_sb[:, ech, st * 512:(st + 1) * 512],
                            start=(ech == 0), stop=(ech == ECH - 1))
                    dest = QT if dch < 4 else KT
                    pair = dch % 4
                    nc.vector.tensor_scalar_add(
                        out=dest[:, pair * S + st * 512:
                                 pair * S + (st + 1) * 512],
                        in0=pq[:], scalar1=bqk_sb[:, dch:dch + 1])
                    yield 40

                def v_chunk(sub):
                    t = st * 4 + sub
                    pv = ps_x.tile([128, 512], f32, tag="px", name="pv")
                    for ech in range(ECH):
                        nc.tensor.matmul(
                            pv[:], xT_sb[:, ech, t * 128:(t + 1) * 128],
                            wv_sb[:, ech, :],
                            start=(ech == 0), stop=(ech == ECH - 1))
                    nc.vector.tensor_add(
                        out=Vv[:, :, t, 0:64],
                        in0=pv[:].rearrange("p (l d) -> p l d", d=64),
                        in1=bv_bc.rearrange("p (l d) -> p l d", d=64))
                    nc.vector.tensor_copy(out=Vv[:, :, t, 64],
                                          in_=ones_f[:, 0:HL])
                    yield 40

                if part is None:
                    part = [("qk", d) for d in range(8)] + \
                           [("v", s) for s in range(4)]
                for kind, i in part:
                    yield from (qk_chunk(i) if kind == "qk" else v_chunk(i))
                yield 1

            def phase1(lp, qt):
                PT = PTs[(qt * 4 + lp) % 2]
                """Energy + exp for head pair lp (heads 2lp, 2lp+1) over one
                q-window. The two heads' energy matmuls share one eps tile
                (rows 0:64 / 64:128 -> disjoint PE row-groups, so they run
                concurrently in the array; 133 vs 229 ns/MM measured); eps
                pools ping-pong per k-tile. exp lands in the persistent PT
                ring (slot = kt), consumed later by this pair's PV sweep,
                which weaves into the NEXT pair's exp stretch as filler —
                so the (in-order) PE queue never parks behind an exp."""
                pair = lp
                q0 = pair * S + qt * 512
                n_kt = 4 * (qt + 1)
                for kt in range(n_kt):
                    pool = eps_pools[kt % 2]
                    eps = pool.tile([128, 2, 512], f32, tag=f"eps{kt % 2}")
                    if eps_fresh[kt % 2] > 0:
                        nc.vector.memset(eps[:], 0.0)
                        eps_fresh[kt % 2] -= 1
                    dj = kt - 4 * qt  # diagonal sub-block index (if >= 0)
                    a = dj * 128 if dj >= 0 else 0
                    if not skip_energy:
                        for h in range(2):
                            pb = h * 64
                            nc.tensor.matmul(
                                eps[:, h, a:],
                                KT[pb:pb + 64, pair * S + kt * 128:
                                   pair * S + (kt + 1) * 128],
                                QT[pb:pb + 64, q0 + a:q0 + 512],
                                start=True, stop=True)
                    yield
                    ao = 448 if act_slim else a
                    if not skip_exp:
                        nc.scalar.activation(out=PT[:, kt, :, ao:],
                                             in_=eps[:, :, ao:],
                                             func=AF.Exp, scale=float(SCALE))
                    if dj >= 0 and not skip_affsel:
                        # keep where q_local - k_local >= 0; only the 128-wide
                        # diagonal block needs masking, both heads share it
                        nc.gpsimd.affine_select(
                            out=PT[:, kt, :, a:a + 128],
                            in_=PT[:, kt, :, a:a + 128],
                            compare_op=mybir.AluOpType.is_ge,
                            fill=0.0, base=0,
                            pattern=[[0, 2], [1, 128]],
                            channel_multiplier=-1)
                    yield

            def pv_sweep(lp, qt):
                """PV + softmax-normalize for pair lp of q-window qt,
                consuming the PT ring kt-outer (so later writers of each PT
                slot unblock early). Per (head, q-chunk of 128): a K=128
                M=128 N=65 matmul with PT as the stationary -> O[q, d|denom]
                accumulates in PSUM (42 vs 229 ns/MM measured). Softmax
                normalize is a per-partition scalar multiply (no PE), and A
                reaches AT-layout via XBAR DMA transposes (DMA is idle)."""
                PT = PTs[(qt * 4 + lp) % 2]
                pair = lp
                q0 = pair * S + qt * 512
                n_kt = 4 * (qt + 1)
                O0 = ps_o.tile([128, 4, 65], f32, tag="oT", name="O0")
                O1 = ps_o.tile([128, 4, 65], f32, tag="oT", name="O1")
                Os = [O0, O1]
                for kt in range(n_kt):
                    dj = kt - 4 * qt
                    for h in range(2):
                        l = 2 * lp + h
                        if skip_pv:
                            yield 1
                            continue
                        for qc in range(4):
                            if dj > qc:
                                continue  # q-chunk fully masked for this kt
                            # start=True zeroes the whole 2KB PSUM bank (the
                            # "zero region"), so exactly one start (first
                            # matmul into the bank) and one stop (last) per
                            # q-window; the other sub-regions accumulate onto
                            # the bank-wide lazy zero.
                            nc.tensor.matmul(
                                Os[h][:, qc, :],
                                PT[:, kt, h, qc * 128:(qc + 1) * 128],
                                V[:, (l * NST + kt) * 65:
                                  (l * NST + kt) * 65 + 65],
                                start=(kt == 0 and qc == 0),
                                stop=(kt == n_kt - 1 and qc == 3))
                            yield 1
                # softmax normalization: O[:, qc, 64] = denominators, per-q on
                # partitions -> plain per-partition scalar ops, no PE needed.
                if skip_norm:
                    yield 1
                    return
                den = p_s.tile([128, 2, 4], f32, tag="den")
                for h in range(2):
                    nc.vector.tensor_copy(out=den[:, h, :],
                                          in_=Os[h][:, :, 64])
                rcp = p_s.tile([128, 2, 4], f32, tag="rcp")
                nc.vector.reciprocal_approx_fast(out=rcp[:], in_=den[:])
                asb = p_s.tile([128, 4, 2, 64], f16, tag="asb")
                for h in range(2):
                    nc.vector.tensor_mul(
                        out=asb[:, :, h, :], in0=Os[h][:, :, 0:64],
                        in1=rcp[:, h, :, None].broadcast_to((128, 4, 64)))
                    yield 1
                if norm_mode == "noat":
                    yield 1
                    return
                for qc in range(4):
                    # XBAR transpose [128 q, 128 dl] -> AT[dl, q] on the DMA
                    nc.sync.dma_start(
                        out=AT[:, q0 + qc * 128:q0 + (qc + 1) * 128],
                        in_=asb[:, qc, :, :], transpose=True)
                yield 1

            def fc_stream(qt):
                """fc_out for q-window qt (row-parallel partial) + out DMA."""
                for st_loc in range(4):
                    st = qt * 4 + st_loc
                    o_sb = p_o.tile([128, E], f16, tag="o_sb")
                    for half in range(2):
                        pf = ps_x.tile([128, 512], f32, tag="px")
                        for dch in range(4):
                            nc.tensor.matmul(
                                pf[:],
                                AT[:, dch * S + st * 128:
                                   dch * S + (st + 1) * 128],
                                wo_sb[:, dch, half * 512:(half + 1) * 512],
                                start=(dch == 0), stop=(dch == 3))
                        nc.vector.tensor_add(
                            out=o_sb[:, half * 512:(half + 1) * 512],
                            in0=pf[:],
                            in1=bo_bc[:, half * 512:(half + 1) * 512])
                        yield 20
                    nc.sync.dma_start(out=out[st * 128:(st + 1) * 128, :],
                                      in_=o_sb[:])
                    yield 1

            PROJ_STEPS = 8 * ECH + 4 * ECH + 1   # 97
            FC_STEPS = 4 * (2 * 4 + 1)           # 36
            LEAD = [("qk", 0), ("qk", 4),
                    ("v", 0), ("v", 1), ("v", 2), ("v", 3)]
            PAIRP = {1: [("qk", 1), ("qk", 5)], 2: [("qk", 2), ("qk", 6)],
                     3: [("qk", 3), ("qk", 7)]}
            proj_done = set()

            def tracked(gen, key):
                yield from gen
                proj_done.add(key)

            # lead-in: only pair-0's Q/K and V for s-window 0; the other
            # pairs' Q/K weave into the qt=0 attention stretch below
            for _ in proj_stream(0, LEAD):
                pass

            if upto < 2:
                for lp in (1, 2, 3):
                    for _ in proj_stream(0, PAIRP[lp]):
                        pass
                for st in range(1, NQT):
                    for _ in proj_stream(st):
                        pass
                ctx.close()
                return

            fillers = deque()
            cur_sweep = [None]   # at most ONE sweep in flight (O-pool order)
            W_BIG = 5            # ~229ns proj/fc matmul vs ~42ns sweep matmul

            def consume_one():
                """Run one filler step; the in-flight sweep has priority
                (PT-ring consumers must not lag behind exp writers).
                Returns the step's pacing weight (0 = nothing left)."""
                if cur_sweep[0] is not None:
                    w = next(cur_sweep[0], "end")
                    if w != "end":
                        return w
                    cur_sweep[0] = None
                while fillers:
                    w = next(fillers[0], "end")
                    if w == "end":
                        fillers.popleft()
                    else:
                        return w
                return 0

            prev_sweep_steps = 0
            for qt in range(NQT):
                n_kt = 4 * (qt + 1)
                sweep_steps = 2 * (16 * qt + 10) + 3
                budget = prev_sweep_steps if cur_sweep[0] is not None else 0
                if qt == 0:
                    for lp2 in (1, 2, 3):
                        fillers.append(tracked(proj_stream(0, PAIRP[lp2]),
                                               (0, lp2)))
                        budget += 2 * 40 + 1
                if qt + 1 < NQT:
                    # next window's pair-0 Q/K must finish by this window's
                    # end; V gates only the first sweep install; the other
                    # pairs' chunks spill into the next window, gated below
                    fillers.append(tracked(proj_stream(qt + 1, LEAD[0:2]),
                                           (qt + 1, 0)))
                    budget += 2 * 40 + 1
                    fillers.append(tracked(proj_stream(qt + 1, LEAD[2:6]),
                                           (qt + 1, "v")))
                    budget += 4 * 40 + 1
                    for lp2 in (1, 2, 3):
                        fillers.append(tracked(proj_stream(qt + 1,
                                                           PAIRP[lp2]),
                                               (qt + 1, lp2)))
                        budget += 2 * 40 + 1
                fc_stage = deque()
                if qt == NQT - 1 and upto >= 3:
                    # one fc stream per pair-stretch, so filler supply lasts
                    # through the whole (longest) window instead of running
                    # dry before the final pair's ACT-bound stretch
                    fc_stage = deque(fc_stream(q2) for q2 in range(NQT - 1))
                total_rounds = 4 * 2 * n_kt
                r = 0
                done_f = 0
                for lp in range(4):
                    if fc_stage and lp >= 1:
                        fillers.append(fc_stage.popleft())
                        budget += FC_STEPS * 5
                    # correctness gate: this pair's Q/K chunks must be fully
                    # emitted before its energy matmuls
                    while lp > 0 and (qt, lp) not in proj_done:
                        if not consume_one():
                            break
                    for _ in phase1(lp, qt):
                        r += 1
                        target = min(budget, budget * (r + 2) // total_rounds)
                        while done_f < target:
                            w = consume_one()
                            if w == 0:
                                break
                            done_f += w
                    # install this pair's sweep (for the next exp stretch);
                    # finish any unconsumed previous sweep first
                    if cur_sweep[0] is not None:
                        while next(cur_sweep[0], "end") != "end":
                            pass
                    # first sweep of a window reads V: V chunks must be
                    # fully emitted first
                    while lp == 0 and qt > 0 and (qt, "v") not in proj_done:
                        if not consume_one():
                            break
                    cur_sweep[0] = pv_sweep(lp, qt)
                    if SERIAL_SWEEP:
                        while next(cur_sweep[0], "end") != "end":
                            pass
                        cur_sweep[0] = None
                    budget += sweep_steps
                # drain only through the next window's pair-0 prefix
                # (Q/K pair0 + V); later pairs' proj chunks keep weaving
                # into the next window's stretches, gated per pair above
                while qt + 1 < NQT and (qt + 1, 0) not in proj_done:
                    if not consume_one():
                        break
                if qt + 1 == NQT:
                    while fillers:
                        if next(fillers[0], "end") == "end":
                            fillers.popleft()
                prev_sweep_steps = sweep_steps
            for g in fc_stage:
                fillers.append(g)
            while consume_one():
                pass
            if upto >= 3:
                for _ in fc_stream(NQT - 1):
                    pass
            ctx.close()

        if loop > 1:
            with tc.For_i(0, loop, 1):
                _rep_body()
        else:
            for _rep in range(reps):
                _rep_body()

    nc.finalize()
    return nc


def _in_maps(x, w_attn, b_attn, w_out, b_out):
    x = np.asarray(x, np.float32)
    w_attn = np.asarray(w_attn, np.float32)
    b_attn = np.asarray(b_attn, np.float32)
    w_out = np.asarray(w_out, np.float32)
    b_out = np.asarray(b_out, np.float32)
    zeros_e = np.zeros((E,), np.float32)

    def chunk_rows(w):
        # [(nch*128), cols] -> [128, nch*cols]: row p holds chunks eo of
        # original rows eo*128+p, so each DMA partition line is contiguous
        nch = w.shape[0] // 128
        return np.ascontiguousarray(
            w.reshape(nch, 128, w.shape[1]).transpose(1, 0, 2)
            .reshape(128, nch * w.shape[1]).astype(np.float16))

    maps = []
    for c in range(8):
        b, hg = c // 2, c % 2
        sq = slice(0 * E + hg * DL, 0 * E + (hg + 1) * DL)
        sk = slice(1 * E + hg * DL, 1 * E + (hg + 1) * DL)
        sv = slice(2 * E + hg * DL, 2 * E + (hg + 1) * DL)
        maps.append({
            "xT": np.ascontiguousarray(x[b].T.astype(np.float16)),
            "wq": chunk_rows(w_attn[:, sq]),
            "wk": chunk_rows(w_attn[:, sk]),
            "wv": chunk_rows(w_attn[:, sv]),
            "wo": chunk_rows(w_out[hg * DL:(hg + 1) * DL, :]),
            "bqk": np.concatenate([b_attn[sq], b_attn[sk]]),
            "bv": np.ascontiguousarray(b_attn[sv]),
            "bo": b_out if hg == 0 else zeros_e,
        })
    return maps


def _run(x, w_attn, b_attn, w_out, b_out, trace=False):
    if "nc" not in _CACHE:
        _CACHE["nc"] = _build()
    maps = _in_maps(x, w_attn, b_attn, w_out, b_out)
    res = run_bass_kernel_spmd(_CACHE["nc"], maps, list(range(8)), trace=trace)
    outs = np.empty((B, S, E), np.float32)
    for b in range(B):
        outs[b] = (res.results[2 * b]["out"].astype(np.float32)
                   + res.results[2 * b + 1]["out"].astype(np.float32))
    return outs, res


def kernel(x, w_attn, b_attn, w_out, b_out):
    outs, _ = _run(x, w_attn, b_attn, w_out, b_out, trace=False)
    return outs

